# revision 1
# baseline (speedup 1.0000x reference)
import sys, os
sys.path.insert(0, "/opt/trn_rl_repo")
import numpy as np
import ml_dtypes

from concourse import bass, bacc, tile, mybir
from concourse.bass_utils import run_bass_kernel_spmd

bf16 = mybir.dt.float16  # fp16: same PE rate as bf16, 3 more mantissa bits
f32 = mybir.dt.float32
i16 = mybir.dt.int16
f8 = mybir.dt.float8e4
AF = mybir.ActivationFunctionType
ALU = mybir.AluOpType
X = mybir.AxisListType.X

NC = 8
H = 128
EPS = 1e-5


def _wrap_idx(a):
    # gather idx layout: token i at [i%16, i//16], replicated to 128 partitions
    n = len(a)
    n16 = (n + 15) // 16
    w = np.zeros((16, n16), np.int16)
    for p in range(16):
        w[p, : len(a[p::16])] = a[p::16]
    return np.tile(w, (8, 1))


def build(cfg):
    N, E, L = cfg["N"], cfg["E"], cfg["L"]
    NPC, NPAD, ECP = cfg["NPC"], cfg["NPAD"], cfg["EC_PAD"]
    wsched = cfg["wsched"]
    NT = NPAD // 128              # node tiles (== agg windows)
    ET = ECP // 128               # edge tiles
    ECH = ECP // 512              # edge chunks (4 tiles each)
    NS = cfg["NS"]                # legacy half split (kept for cfg compat)
    NT_ = NPAD // 128
    # 3-way node split by window groups (~30/40/30% of edge tiles)
    NS1 = cfg["NS1"]
    NS2 = cfg["NS2"]
    TBL = NC * NPAD
    fl = cfg["flags"]
    KL = int(os.environ.get("KLAYERS", L))


    # per-tile window + first/last-in-window flags
    fw = [False] * ET
    lw = [False] * ET
    seen = set()
    for t in range(ET):
        if wsched[t] not in seen:
            seen.add(wsched[t])
            fw[t] = True
    seen = set()
    for t in range(ET - 1, -1, -1):
        if wsched[t] not in seen:
            seen.add(wsched[t])
            lw[t] = True
    # last chunk index containing a tile of each window group
    def lastc(nw):
        return max(c for c in range(ECH)
                   if any(wsched[4 * c + j] < nw for j in range(4))) if nw > 0 else -1
    splitc1 = lastc(NS1 // 128)
    splitc2 = lastc(NS2 // 128)

    nc = bacc.Bacc(None, target_bir_lowering=False, num_devices=NC)
    P = lambda n_, s, d: nc.declare_dram_parameter(n_, s, d, isOutput=False)

    xT_d = P("xT", [5, NPAD], bf16)
    eaT_d = P("eaT", [3, ECP], bf16)
    src_d = P("srci", [128, ECP // 16], i16)
    seg2_d = P("seg2", [ECH, 128, 1024], bf16)
    icnt_d = P("icnt", [128, NT], f32)
    ident_d = P("ident", [128, 128], bf16)
    identf_d = P("identf", [128, 128], f32)
    identq_d = P("identq", [128, 128], f8)
    ones1_d = P("ones1", [1, 128], f32)
    onesK_d = P("onesK", [128, 1], f32)
    encNW0_d = P("encNW0", [5, 128], bf16)
    encNW_d = P("encNW", [3, 128, 128], bf16)
    encEW0_d = P("encEW0", [3, 128], bf16)
    encEW_d = P("encEW", [3, 128, 128], bf16)
    eW0_d = P("eW0", [L, 3, 128, 128], bf16)
    eWs0_d = P("eWs0", [L, 128, 128], bf16)
    eWs1_d = P("eWs1", [L, 128, 128], bf16)
    nW0_d = P("nW0", [L, 2, 128, 128], bf16)
    nWs0_d = P("nWs0", [L, 128, 128], bf16)
    nWs1_d = P("nWs1", [L, 128, 128], bf16)
    decW_d = P("decW", [3, 128, 128], bf16)
    decWl_d = P("decWl", [128, 3], bf16)
    vE_d = P("vE", [128, L], bf16)
    vN_d = P("vN", [128, L], bf16)
    GE_d = P("GE", [L, 128, 128], bf16)
    GN_d = P("GN", [L, 128, 128], bf16)
    onesh_d = P("onesh", [128, 1], bf16)
    encNb_d = P("encNb", [128, 4], f32)
    encEb_d = P("encEb", [128, 4], f32)
    eb_d = P("eb", [128, 3 * L], f32)
    nb_d = P("nb", [128, 3 * L], f32)
    decb_d = P("decb", [128, 3], f32)
    if fl["eln"]:
        elnw_d = P("elnw", [L, 128, 128], f32)
        elnb_d = P("elnb", [L, 128, 128], f32)
    if fl["nln"]:
        nlnw_d = P("nlnw", [L, 128, 128], f32)
        nlnb_d = P("nlnb", [L, 128, 128], f32)
    if fl["gln"]:
        gNw_d = P("gNw", [128, 1], f32)
        gNb_d = P("gNb", [128, 1], f32)
        gEw_d = P("gEw", [128, 1], f32)
        gEb_d = P("gEb", [128, 1], f32)
    if fl["decbl"]:
        decbl_d = P("decbl", [128, 3], f32)

    out_d = nc.declare_dram_parameter("out", [NPAD, 3], f32, isOutput=True)
    zshA_d = nc.dram_tensor("zshA", [NS1, 128], bf16)
    zshM_d = nc.dram_tensor("zshM", [NS2 - NS1, 128], bf16)
    zshB_d = nc.dram_tensor("zshB", [NPAD - NS2, 128], bf16)
    zshard = [(0, NS1, zshA_d), (NS1, NS2, zshM_d), (NS2, NPAD, zshB_d)]
    ztab0_d = nc.dram_tensor("ztab0", [TBL, 128], bf16, addr_space="Shared")
    ztabA_d = nc.dram_tensor("ztabA", [TBL, 128], bf16, addr_space="Shared")
    ztabB_d = nc.dram_tensor("ztabB", [TBL, 128], bf16, addr_space="Shared")
    ztq = [ztabA_d, ztabB_d]
    sti_d = nc.dram_tensor("sti", [4], f32)
    sto_d = nc.dram_tensor("sto", [4], f32, addr_space="Shared")
    RG = [list(range(NC))]

    with tile.TileContext(nc) as tc:
        with (
            tc.tile_pool(name="const", bufs=1) as cp,
            tc.tile_pool(name="big", bufs=1) as bigp,
            tc.tile_pool(name="seg", bufs=3) as segp,
            tc.tile_pool(name="work", bufs=3) as wp,
            tc.tile_pool(name="stat", bufs=6) as sp,
            tc.tile_pool(name="pA", bufs=2, space="PSUM") as pA,
            tc.tile_pool(name="p3", bufs=2, space="PSUM") as p3,
            tc.tile_pool(name="pT", bufs=2, space="PSUM") as pT,
            tc.tile_pool(name="pG", bufs=2, space="PSUM") as pG,
        ):
            # ---- persistent SBUF ----
            e_fm = bigp.tile([128, ET, 128], bf16)      # e features, feature-major (fp16)
            h_own = bigp.tile([128, NT, 128], f32)      # node features f32, fm
            h_bf = bigp.tile([128, NT, 128], bf16)      # bf16 copy of h (raw pre-AR1)
            zdst_nm = bigp.tile([128, NT, 128], bf16)   # W1dst.T h, node-major
            agg_fm = bigp.tile([128, NT, 128], bf16)    # aggregated messages, fm
            accE = bigp.tile([128, ECH], f32)           # per-chunk e-enc sums
            accEq = bigp.tile([128, ECH], f32)
            NCH = (NPAD + 511) // 512
            accN = bigp.tile([128, NCH], f32)
            accNq = bigp.tile([128, NCH], f32)

            # ---- constants ----
            def ld(shape, dt, src, tag):
                t = cp.tile(shape, dt, tag=tag)
                nc.sync.dma_start(t[:], src[:])
                return t

            def ldw(src, n_, tag, dt=bf16):
                t = cp.tile([128, n_, 128], dt, tag=tag)
                nc.sync.dma_start(t[:], src[:].rearrange("n k m -> k n m"))
                return t

            xT = ld([5, NPAD], bf16, xT_d, "xT")
            srci = ld([128, ECP // 16], i16, src_d, "srci")
            icnt = ld([128, NT], f32, icnt_d, "icnt")
            ident = ld([128, 128], bf16, ident_d, "ident")
            identf = ld([128, 128], f32, identf_d, "identf")
            identq = ld([128, 128], f8, identq_d, "identq")
            ones1 = ld([1, 128], f32, ones1_d, "ones1")
            onesK = ld([128, 1], f32, onesK_d, "onesK")
            encNW0 = ld([5, 128], bf16, encNW0_d, "encNW0")
            encNW = ldw(encNW_d, 3, "encNW")
            encEW0 = ld([3, 128], bf16, encEW0_d, "encEW0")
            encEW = ldw(encEW_d, 3, "encEW")
            eW0 = cp.tile([128, L * 3, 128], bf16, tag="eW0")
            nc.sync.dma_start(eW0[:], eW0_d[:].rearrange("l n k m -> k (l n) m"))
            eWs0 = ldw(eWs0_d, L, "eWs0")
            eWs1 = ldw(eWs1_d, L, "eWs1")
            nW0 = cp.tile([128, L * 2, 128], bf16, tag="nW0")
            nc.sync.dma_start(nW0[:], nW0_d[:].rearrange("l n k m -> k (l n) m"))
            nWs0 = ldw(nWs0_d, L, "nWs0")
            nWs1 = ldw(nWs1_d, L, "nWs1")
            decW = ldw(decW_d, 3, "decW")
            decWl = ld([128, 3], bf16, decWl_d, "decWl")
            vE = ld([128, L], bf16, vE_d, "vE")
            vN = ld([128, L], bf16, vN_d, "vN")
            GE = ldw(GE_d, L, "GE")
            GN = ldw(GN_d, L, "GN")
            onesh = ld([128, 1], bf16, onesh_d, "onesh")
            encNb = ld([128, 4], f32, encNb_d, "encNb")
            encEb = ld([128, 4], f32, encEb_d, "encEb")
            eb = ld([128, 3 * L], f32, eb_d, "eb")
            nb = ld([128, 3 * L], f32, nb_d, "nb")
            decb = ld([128, 3], f32, decb_d, "decb")
            if fl["eln"]:
                elnw = ldw(elnw_d, L, "elnw", f32)
                elnb = ldw(elnb_d, L, "elnb", f32)
            if fl["nln"]:
                nlnw = ldw(nlnw_d, L, "nlnw", f32)
                nlnb = ldw(nlnb_d, L, "nlnb", f32)
            if fl["gln"]:
                gNw = ld([128, 1], f32, gNw_d, "gNw")
                gNb = ld([128, 1], f32, gNb_d, "gNb")
                gEw = ld([128, 1], f32, gEw_d, "gEw")
                gEb = ld([128, 1], f32, gEb_d, "gEb")
            if fl["decbl"]:
                decbl = ld([128, 3], f32, decbl_d, "decbl")
            epsA = sp.tile([128, 1], f32, tag="epsA")
            nc.vector.memset(epsA[:], EPS)

            # 4-linear encoder MLP, feature-major; returns final psum [128,w]
            def enc_mlp(rhs, W0, Wm, b):
                w = rhs.shape[-1]
                ps = pA.tile([128, 512], f32, tag="pA")
                nc.tensor.matmul(ps[:, :w], W0, rhs, start=True, stop=True)
                a1 = wp.tile([128, 512], bf16, tag="a1")
                nc.vector.tensor_scalar(a1[:, :w], ps[:, :w], b[:, 0:1], 0.0, ALU.add, ALU.max)
                ps2 = p3.tile([128, 512], f32, tag="p3", name="ps2e")
                nc.tensor.matmul(ps2[:, :w], Wm[:, 0, :], a1[:, :w], start=True, stop=True)
                a2 = wp.tile([128, 512], bf16, tag="a2")
                nc.vector.tensor_scalar(a2[:, :w], ps2[:, :w], b[:, 1:2], 0.0, ALU.add, ALU.max)
                ps3 = pA.tile([128, 512], f32, tag="pA")
                nc.tensor.matmul(ps3[:, :w], Wm[:, 1, :], a2[:, :w], start=True, stop=True)
                a3 = wp.tile([128, 512], bf16, tag="a3")
                nc.scalar.activation(a3[:, :w], ps3[:, :w], AF.Relu, bias=b[:, 2:3])
                ps4 = p3.tile([128, 512], f32, tag="p3", name="ps4e")
                nc.tensor.matmul(ps4[:, :w], Wm[:, 2, :], a3[:, :w], start=True, stop=True)
                return ps4

            # LN row stats; s1 via per-tile matmul against weight-rowsum vcol
            def ln_stats(ps3, nt, flat_w, a2t, vcol, psS):
                st = sp.tile([128, 8], f32, tag="st")
                for j in range(nt):
                    nc.tensor.matmul(psS[:, j:j + 1], a2t[:, j * 128:(j + 1) * 128], vcol, start=True, stop=True)
                nc.vector.tensor_copy(st[:, 0:nt], psS[:, 0:nt])
                scr = wp.tile([128, 512], bf16, tag="scr")
                ps3f = ps3[:].rearrange("p a b -> p (a b)")
                nc.scalar.activation(scr[:, :flat_w], ps3f[:, :flat_w], AF.Square)
                nc.vector.tensor_reduce(st[:, 4:4 + nt], scr[:].rearrange("p (a b) -> p a b", a=4)[:, 0:nt, :], X, ALU.add)
                t2 = sp.tile([128, 4], f32, tag="mu2")
                nc.vector.tensor_tensor(t2[:, 0:nt], st[:, 0:nt], st[:, 0:nt], ALU.mult)
                t3 = sp.tile([128, 4], f32, tag="var")
                nc.vector.scalar_tensor_tensor(t3[:, 0:nt], t2[:, 0:nt], -1.0 / 128, st[:, 4:4 + nt], ALU.mult, ALU.add)
                sd = sp.tile([128, 4], f32, tag="sd")
                nc.scalar.activation(sd[:, 0:nt], t3[:, 0:nt], AF.Sqrt, bias=epsA[:], scale=1.0 / 128)
                rs = sp.tile([128, 4], f32, tag="rs")
                nc.vector.reciprocal(rs[:, 0:nt], sd[:, 0:nt])
                nmr = sp.tile([128, 4], f32, tag="nmr")
                nc.vector.scalar_tensor_tensor(nmr[:, 0:nt], st[:, 0:nt], -1.0 / 128, rs[:, 0:nt], ALU.mult, ALU.mult)
                return rs, nmr

            # ================= NODE ENCODER =================
            for c in range(NCH):
                c0 = c * 512
                w = min(512, NPAD - c0)
                nt = w // 128
                ps4 = enc_mlp(xT[:, c0:c0 + w], encNW0[:], encNW, encNb)
                hv = h_own[:].rearrange("p a b -> p (a b)")[:, c0:c0 + w]
                nc.scalar.activation(hv, ps4[:, :w], AF.Copy, accum_out=accN[:, c:c + 1])
                scr = wp.tile([128, 512], f32, tag="scr")
                nc.scalar.activation(scr[:, :w], ps4[:, :w], AF.Square)
                nc.vector.tensor_reduce(accNq[:, c:c + 1], scr[:, :w], X, ALU.add)
                nc.gpsimd.tensor_copy(h_bf[:].rearrange("p a b -> p (a b)")[:, c0:c0 + w], hv)

            # z0 tables from raw h (pre graph-LN); valid when gln scale is uniform
            if not fl["gln"]:
                for c in range(NCH):
                    nt = min(4, NT - c * 4)
                    pz = p3.tile([128, 4, 128], f32, tag="p3")
                    pz2 = p3.tile([128, 4, 128], f32, tag="p3")
                    for j in range(nt):
                        t = c * 4 + j
                        nc.tensor.matmul(pz[:, j, :], h_bf[:, t, :], eW0[:, 1, :], start=True, stop=True)
                        nc.tensor.matmul(pz2[:, j, :], h_bf[:, t, :], eW0[:, 0, :], start=True, stop=True)
                    zr = wp.tile([128, 4, 128], bf16, tag="zr")
                    nc.vector.tensor_copy(zr[:, 0:nt, :], pz[:, 0:nt, :])
                    nc.vector.tensor_copy(zdst_nm[:, c * 4:c * 4 + nt, :], pz2[:, 0:nt, :])
                    for j in range(nt):
                        t = c * 4 + j
                        r0_ = t * 128
                        for lo_, hi_, zt_ in zshard:
                            if lo_ <= r0_ < hi_:
                                nc.sync.dma_start(zt_[r0_ - lo_:r0_ - lo_ + 128, :], zr[:, j, :])

            # ================= EDGE ENCODER =================
            for c in range(ECH):
                c0 = c * 512
                eat = wp.tile([3, 512], bf16, tag="eat")
                nc.sync.dma_start(eat[:], eaT_d[:, c0:c0 + 512])
                ps4 = enc_mlp(eat[:], encEW0[:], encEW, encEb)
                ev = e_fm[:].rearrange("p a b -> p (a b)")[:, c0:c0 + 512]
                nc.scalar.activation(ev, ps4[:], AF.Copy, accum_out=accE[:, c:c + 1])
                scr = wp.tile([128, 512], f32, tag="scr")
                nc.scalar.activation(scr[:], ps4[:], AF.Square)
                nc.vector.tensor_reduce(accEq[:, c:c + 1], scr[:], X, ALU.add)

            # combined graph-stats AllReduce (h + e)
            st4 = sp.tile([128, 4], f32, tag="st4")
            nc.vector.tensor_reduce(st4[:, 0:1], accN[:, 0:NCH], X, ALU.add)
            nc.vector.tensor_reduce(st4[:, 1:2], accNq[:, 0:NCH], X, ALU.add)
            nc.vector.tensor_reduce(st4[:, 2:3], accE[:, 0:ECH], X, ALU.add)
            nc.vector.tensor_reduce(st4[:, 3:4], accEq[:, 0:ECH], X, ALU.add)
            psst = p3.tile([128, 4, 128], f32, tag="p3", name="psst")
            nc.tensor.matmul(psst[:4, 0, :1], st4[:], onesK[:], start=True, stop=True)
            stv = sp.tile([4, 1], f32, tag="stv")
            nc.scalar.activation(stv[:], psst[:4, 0, :1], AF.Copy)
            nc.sync.dma_start(sti_d[:], stv[:, 0:1])
            nc.gpsimd.collective_compute("AllReduce", ALU.add, replica_groups=RG,
                                         ins=[sti_d[:]], outs=[sto_d[:]])
            if not fl["gln"]:
                nc.gpsimd.collective_compute("AllGather", ALU.bypass, replica_groups=RG,
                                             ins=[zshA_d[:]], outs=[ztab0_d[0:NC * NS1]])
                nc.gpsimd.collective_compute("AllGather", ALU.bypass, replica_groups=RG,
                                             ins=[zshM_d[:]], outs=[ztab0_d[NC * NS1:NC * NS2]])
                nc.gpsimd.collective_compute("AllGather", ALU.bypass, replica_groups=RG,
                                             ins=[zshB_d[:]], outs=[ztab0_d[NC * NS2:TBL]])

            # graph-LN factors: a = w/(std+eps), c = b - mu*a  (w=1,b=0 unless gln)
            def g_factors(sto, count, gw, gb, tagp):
                st14 = sp.tile([1, 2], f32, tag="g14" + tagp)
                nc.sync.dma_start(st14[:], sto)
                psb = p3.tile([128, 4, 128], f32, tag="p3", name="psb")
                nc.tensor.matmul(psb[:, 0, :2], ones1[:], st14[:], start=True, stop=True)
                stb = sp.tile([128, 2], f32, tag="gstb" + tagp)
                nc.scalar.activation(stb[:], psb[:, 0, :2], AF.Copy)
                mu = sp.tile([128, 1], f32, tag="gmu" + tagp)
                nc.vector.tensor_scalar(mu[:], stb[:, 0:1], 1.0 / count, None, ALU.mult)
                e2 = sp.tile([128, 1], f32, tag="ge2" + tagp)
                nc.vector.tensor_scalar(e2[:], stb[:, 1:2], 1.0 / count, None, ALU.mult)
                mu2 = sp.tile([128, 1], f32, tag="gmu2" + tagp)
                nc.vector.tensor_tensor(mu2[:], mu[:], mu[:], ALU.mult)
                var = sp.tile([128, 1], f32, tag="gvar" + tagp)
                nc.vector.tensor_tensor(var[:], e2[:], mu2[:], ALU.subtract)
                sd = sp.tile([128, 1], f32, tag="gsd" + tagp)
                nc.scalar.activation(sd[:], var[:], AF.Sqrt)
                nc.vector.tensor_scalar(sd[:], sd[:], EPS, None, ALU.add)
                a = sp.tile([128, 1], f32, tag="ga" + tagp)
                nc.vector.reciprocal(a[:], sd[:])
                if gw is not None:
                    nc.vector.tensor_tensor(a[:], a[:], gw[:], ALU.mult)
                cc = sp.tile([128, 1], f32, tag="gc" + tagp)
                nc.vector.tensor_scalar(cc[:], mu[:], a[:], -1.0, ALU.mult, ALU.mult)
                if gb is not None:
                    nc.vector.tensor_tensor(cc[:], cc[:], gb[:], ALU.add)
                return a, cc

            a_h, c_h = g_factors(sto_d[0:2], float(N) * H, gNw if fl["gln"] else None,
                                 gNb if fl["gln"] else None, "h")
            a_e, c_e = g_factors(sto_d[2:4], float(E) * H, gEw if fl["gln"] else None,
                                 gEb if fl["gln"] else None, "e")

            # apply h graph-LN; refresh h_bf; scale z tables (uniform a_h)
            for t in range(NT):
                nc.vector.tensor_scalar(h_own[:, t, :], h_own[:, t, :], a_h[:], c_h[:], ALU.mult, ALU.add)
                nc.vector.tensor_copy(h_bf[:, t, :], h_own[:, t, :])
            if not fl["gln"]:
                for t in range(NT):
                    nc.vector.tensor_scalar(zdst_nm[:, t, :], zdst_nm[:, t, :], a_h[:], None, ALU.mult)
            else:
                # gln: rebuild z tables from normalized h (late allgather)
                for c in range(NCH):
                    nt = min(4, NT - c * 4)
                    pz = p3.tile([128, 4, 128], f32, tag="p3")
                    pz2 = p3.tile([128, 4, 128], f32, tag="p3")
                    for j in range(nt):
                        t = c * 4 + j
                        nc.tensor.matmul(pz[:, j, :], h_bf[:, t, :], eW0[:, 1, :], start=True, stop=True)
                        nc.tensor.matmul(pz2[:, j, :], h_bf[:, t, :], eW0[:, 0, :], start=True, stop=True)
                    zr = wp.tile([128, 4, 128], bf16, tag="zr")
                    nc.vector.tensor_copy(zr[:, 0:nt, :], pz[:, 0:nt, :])
                    nc.vector.tensor_copy(zdst_nm[:, c * 4:c * 4 + nt, :], pz2[:, 0:nt, :])
                    for j in range(nt):
                        t = c * 4 + j
                        r0_ = t * 128
                        for lo_, hi_, zt_ in zshard:
                            if lo_ <= r0_ < hi_:
                                nc.sync.dma_start(zt_[r0_ - lo_:r0_ - lo_ + 128, :], zr[:, j, :])
                nc.gpsimd.collective_compute("AllGather", ALU.bypass, replica_groups=RG,
                                             ins=[zshA_d[:]], outs=[ztab0_d[0:NC * NS1]])
                nc.gpsimd.collective_compute("AllGather", ALU.bypass, replica_groups=RG,
                                             ins=[zshM_d[:]], outs=[ztab0_d[NC * NS1:NC * NS2]])
                nc.gpsimd.collective_compute("AllGather", ALU.bypass, replica_groups=RG,
                                             ins=[zshB_d[:]], outs=[ztab0_d[NC * NS2:TBL]])

            # layer-0 e-LN fold: W1e' = a_e (x) W1e ; b1' = eb0 + W1e.T c_e + (W1src+W1dst).T c_h
            w1e0 = cp.tile([128, 128], bf16, tag="w1e0")
            nc.vector.tensor_scalar(w1e0[:], eW0[:, 2, :], a_e[:], None, ALU.mult)
            ceb = sp.tile([128, 1], bf16, tag="ceb")
            nc.vector.tensor_copy(ceb[:], c_e[:])
            chb = sp.tile([128, 1], bf16, tag="chb")
            nc.vector.tensor_copy(chb[:], c_h[:])
            psb1 = p3.tile([128, 4, 128], f32, tag="p3", name="psb1")
            if not fl["gln"]:
                # c_h folded here only when z tables carry raw*a_h (uniform path);
                # gln path rebuilds z from normalized h so no c_h term needed
                nc.tensor.matmul(psb1[:, 0, 0:1], eW0[:, 2, :], ceb[:], start=True, stop=False)
                nc.tensor.matmul(psb1[:, 0, 0:1], eW0[:, 0, :], chb[:], start=False, stop=False)
                nc.tensor.matmul(psb1[:, 0, 0:1], eW0[:, 1, :], chb[:], start=False, stop=True)
            else:
                nc.tensor.matmul(psb1[:, 0, 0:1], eW0[:, 2, :], ceb[:], start=True, stop=True)
            b1f0 = sp.tile([128, 1], f32, tag="b1f0")
            nc.vector.tensor_tensor(b1f0[:], eb[:, 0:1], psb1[:, 0, 0:1], ALU.add)

            # ================= DECODER =================
            def dec_part(r0, r1):
              c0 = r0
              while c0 < r1:
                w = min(512, r1 - c0)
                nt = w // 128
                hv = h_bf[:].rearrange("p a b -> p (a b)")[:, c0:c0 + w]
                ps = pA.tile([128, 512], f32, tag="pA")
                nc.tensor.matmul(ps[:, :w], decW[:, 0, :], hv, start=True, stop=True)
                a1 = wp.tile([128, 512], bf16, tag="a1")
                nc.scalar.activation(a1[:, :w], ps[:, :w], AF.Relu, bias=decb[:, 0:1])
                ps2 = pA.tile([128, 512], f32, tag="pA")
                nc.tensor.matmul(ps2[:, :w], decW[:, 1, :], a1[:, :w], start=True, stop=True)
                a2 = wp.tile([128, 512], bf16, tag="a2")
                nc.scalar.activation(a2[:, :w], ps2[:, :w], AF.Relu, bias=decb[:, 1:2])
                ps3 = pA.tile([128, 512], f32, tag="pA")
                nc.tensor.matmul(ps3[:, :w], decW[:, 2, :], a2[:, :w], start=True, stop=True)
                a3 = wp.tile([128, 512], bf16, tag="a1")
                nc.scalar.activation(a3[:, :w], ps3[:, :w], AF.Relu, bias=decb[:, 2:3])
                pd = p3.tile([128, 4, 128], f32, tag="p3")
                for j in range(nt):
                    nc.tensor.matmul(pd[:, j, :3], a3[:, j * 128:(j + 1) * 128], decWl[:], start=True, stop=True)
                ot = wp.tile([128, 4, 3], f32, tag="ot")
                nc.vector.tensor_copy(ot[:, 0:nt, :], pd[:, 0:nt, 0:3])
                if fl["decbl"]:
                    for j in range(nt):
                        nc.vector.tensor_tensor(ot[:, j, :], ot[:, j, :], decbl[:], ALU.add)
                for j in range(nt):
                    t = c0 // 128 + j
                    nc.sync.dma_start(out_d[t * 128:(t + 1) * 128, :], ot[:, j, :])
                c0 += w

            # ================= MESSAGE-PASSING LAYERS =================
            def node_phase(l, r0, r1, last_layer):
                """process node rows [r0, r1); update h, build z_{l+1} tables"""
                c0 = r0
                while c0 < r1:
                    w = min(512, r1 - c0)
                    nt = w // 128
                    t0 = c0 // 128
                    hv = h_bf[:].rearrange("p a b -> p (a b)")[:, c0:c0 + w]
                    av = agg_fm[:].rearrange("p a b -> p (a b)")[:, c0:c0 + w]
                    ps = pA.tile([128, 512], f32, tag="pA")
                    nc.tensor.matmul(ps[:, :w], nW0[:, 2 * l, :], hv, start=True, stop=False)
                    nc.tensor.matmul(ps[:, :w], nW0[:, 2 * l + 1, :], av, start=False, stop=True)
                    a1 = wp.tile([128, 512], bf16, tag="a1")
                    nc.scalar.activation(a1[:, :w], ps[:, :w], AF.Relu, bias=nb[:, 3 * l:3 * l + 1])
                    ps2 = pA.tile([128, 512], f32, tag="pA")
                    nc.tensor.matmul(ps2[:, :w], nWs0[:, l, :], a1[:, :w], start=True, stop=True)
                    a2 = wp.tile([128, 512], bf16, tag="a2")
                    nc.scalar.activation(a2[:, :w], ps2[:, :w], AF.Relu, bias=nb[:, 3 * l + 1:3 * l + 2])
                    ps3 = p3.tile([128, 4, 128], f32, tag="p3")
                    for j in range(nt):
                        nc.tensor.matmul(ps3[:, j, :], a2[:, j * 128:(j + 1) * 128], nWs1[:, l, :], start=True, stop=True)
                    rs, nmr = ln_stats(ps3, nt, w, a2[:], vN[:, l:l + 1], ps2[:, 0:4])
                    updb = wp.tile([128, 4, 128], bf16, tag="updb")
                    for j in range(nt):
                        nc.scalar.activation(updb[:, j, :], ps3[:, j, :], AF.Identity,
                                             bias=nmr[:, j:j + 1], scale=rs[:, j:j + 1])
                        if fl["nln"]:
                            nc.vector.tensor_tensor(updb[:, j, :], updb[:, j, :], nlnw[:, l, :], ALU.mult)
                            nc.vector.tensor_tensor(updb[:, j, :], updb[:, j, :], nlnb[:, l, :], ALU.add)
                    ptn = pT.tile([128, 4, 128], f32, tag="pT", name="ptn")
                    for j in range(nt):
                        nc.tensor.matmul(ptn[:, j, :], updb[:, j, :], ident[:], start=True, stop=True)
                    hvo = h_own[:].rearrange("p a b -> p (a b)")[:, c0:c0 + w]
                    nc.vector.tensor_tensor(hvo, hvo, ptn[:].rearrange("p a b -> p (a b)")[:, :w], ALU.add)
                    nc.scalar.activation(hv, hvo, AF.Copy)
                    if not last_layer:
                        pz = p3.tile([128, 4, 128], f32, tag="p3")
                        pz2 = p3.tile([128, 4, 128], f32, tag="p3")
                        for j in range(nt):
                            t = t0 + j
                            nc.tensor.matmul(pz[:, j, :], h_bf[:, t, :], eW0[:, 3 * (l + 1) + 1, :], start=True, stop=True)
                            nc.tensor.matmul(pz2[:, j, :], h_bf[:, t, :], eW0[:, 3 * (l + 1) + 0, :], start=True, stop=True)
                        zr = wp.tile([128, 4, 128], bf16, tag="zr")
                        nc.vector.tensor_copy(zr[:, 0:nt, :], pz[:, 0:nt, :])
                        nc.vector.tensor_copy(zdst_nm[:, t0:t0 + nt, :], pz2[:, 0:nt, :])
                        for j in range(nt):
                            t = t0 + j
                            r0_ = t * 128
                            for lo_, hi_, zt_ in zshard:
                                if lo_ <= r0_ < hi_:
                                    nc.sync.dma_start(zt_[r0_ - lo_:r0_ - lo_ + 128, :], zr[:, j, :])
                    c0 += w

            aggps = {}
            for l in range(KL):
                lay0 = (l == 0) and not fl["gln"]
                for c in range(ECH):
                    c0 = c * 512
                    s2c = segp.tile([128, 2, 4, 128], bf16, tag="s2c")
                    nc.sync.dma_start(s2c[:], seg2_d[c].rearrange("p (a b m) -> p a b m", a=2, b=4))
                    zg = wp.tile([128, 4, 128], bf16, tag="zg", bufs=12)
                    zsrc_tab = ztab0_d if l == 0 else ztq[l % 2]
                    nc.gpsimd.dma_gather(zg[:], zsrc_tab[:], srci[:, c * 32:(c + 1) * 32],
                                         512, 512, 128, transpose=False)
                    if lay0:
                        nc.vector.tensor_scalar(zg[:], zg[:], a_h[:], None, ALU.mult)

                    # ps1 = W1e.T e  +  zdst  +  zsrc
                    ps = pA.tile([128, 512], f32, tag="pA")
                    w1e = w1e0 if lay0 else eW0[:, 3 * l + 2, :]
                    for j in range(4):
                        t = 4 * c + j
                        psj = ps[:, j * 128:(j + 1) * 128]
                        nc.tensor.matmul(psj, w1e, e_fm[:, 4 * c + j, :], start=True, stop=False)
                        nc.tensor.matmul(psj, zdst_nm[:, wsched[t], :], s2c[:, 1, j, :], start=False, stop=False)
                        nc.tensor.matmul(psj, zg[:, j, :], ident[:], start=False, stop=True)
                    a1 = wp.tile([128, 512], bf16, tag="a1")
                    nc.scalar.activation(a1[:], ps[:], AF.Relu,
                                         bias=b1f0[:] if lay0 else eb[:, 3 * l:3 * l + 1])
                    ps2 = pA.tile([128, 512], f32, tag="pA")
                    nc.tensor.matmul(ps2[:], eWs0[:, l, :], a1[:], start=True, stop=True)
                    a2 = wp.tile([128, 512], bf16, tag="a2")
                    nc.scalar.activation(a2[:], ps2[:], AF.Relu, bias=eb[:, 3 * l + 1:3 * l + 2])
                    ps3 = p3.tile([128, 4, 128], f32, tag="p3")
                    for j in range(4):
                        nc.tensor.matmul(ps3[:, j, :], a2[:, j * 128:(j + 1) * 128], eWs1[:, l, :], start=True, stop=True)
                    rs, nmr = ln_stats(ps3, 4, 512, a2[:], vE[:, l:l + 1], ps2[:, 0:4])
                    tmpb = wp.tile([128, 4, 128], bf16, tag="tmpb")
                    nsc = 2 if c % 2 == 0 else 1
                    for j in range(4):
                        if j < nsc:
                            nc.scalar.activation(tmpb[:, j, :], ps3[:, j, :], AF.Identity,
                                                 bias=nmr[:, j:j + 1], scale=rs[:, j:j + 1])
                        else:
                            nc.vector.tensor_scalar(tmpb[:, j, :], ps3[:, j, :], rs[:, j:j + 1],
                                                    nmr[:, j:j + 1], ALU.mult, ALU.add)
                        if fl["eln"]:
                            nc.vector.tensor_tensor(tmpb[:, j, :], tmpb[:, j, :], elnw[:, l, :], ALU.mult)
                            nc.vector.tensor_tensor(tmpb[:, j, :], tmpb[:, j, :], elnb[:, l, :], ALU.add)
                    # scatter into window psum accumulators
                    for j in range(4):
                        t = 4 * c + j
                        wd = wsched[t]
                        if fw[t]:
                            aggps[wd] = pG.tile([128, 128], f32, tag="pG", name="aggw")
                        nc.tensor.matmul(aggps[wd][:], s2c[:, 0, j, :], tmpb[:, j, :],
                                         start=fw[t], stop=lw[t])
                        if lw[t]:
                            agf = wp.tile([128, 128], bf16, tag="agf")
                            nc.vector.tensor_scalar(agf[:], aggps[wd][:], icnt[:, wd:wd + 1], None, ALU.mult)
                            pag = pT.tile([128, 4, 128], f32, tag="pT", name="pag")
                            nc.tensor.matmul(pag[:, 0, :], agf[:], ident[:], start=True, stop=True)
                            nc.vector.tensor_copy(agg_fm[:, wd, :], pag[:, 0, :])
                            del aggps[wd]
                    # residual update of e (feature-major master)
                    ptr = pT.tile([128, 4, 128], f32, tag="pT")
                    for j in range(4):
                        nc.tensor.matmul(ptr[:, j, :], tmpb[:, j, :], ident[:], start=True, stop=True)
                    ev = e_fm[:].rearrange("p a b -> p (a b)")[:, c0:c0 + 512]
                    if lay0:
                        nc.vector.tensor_scalar(ev, ev, a_e[:], c_e[:], ALU.mult, ALU.add)
                    nc.vector.tensor_tensor(ev, ev, ptr[:].rearrange("p a b -> p (a b)"), ALU.add)
                    # pipelined node phases at window-group boundaries;
                    # AGs issued a few chunks later so they don't park
                    if KL == L and c == splitc1:
                        node_phase(l, 0, NS1, l == L - 1)
                    if KL == L and c == min(splitc1 + 8, ECH - 2) and l < L - 1:
                        nc.gpsimd.collective_compute("AllGather", ALU.bypass, replica_groups=RG,
                                                     ins=[zshA_d[:]], outs=[ztq[(l + 1) % 2][0:NC * NS1]])
                    if KL == L and c == splitc2:
                        node_phase(l, NS1, NS2, l == L - 1)
                    if KL == L and c == min(splitc2 + 8, ECH - 1) and l < L - 1:
                        nc.gpsimd.collective_compute("AllGather", ALU.bypass, replica_groups=RG,
                                                     ins=[zshM_d[:]], outs=[ztq[(l + 1) % 2][NC * NS1:NC * NS2]])
                if KL == L:
                    node_phase(l, NS2, NPAD, l == L - 1)
                    if l < L - 1:
                        nc.gpsimd.collective_compute("AllGather", ALU.bypass, replica_groups=RG,
                                                     ins=[zshB_d[:]], outs=[ztq[(l + 1) % 2][NC * NS2:TBL]])
                else:
                    node_phase(l, 0, NPAD, l == KL - 1)

            dec_part(0, NPAD)


    nc.compile()
    return nc


def _prep(inputs, cfg):
    N, E, L = cfg["N"], cfg["E"], cfg["L"]
    NPC, NPAD, ECP = cfg["NPC"], cfg["NPAD"], cfg["EC_PAD"]
    NS = cfg["NS"]
    wsched = cfg["wsched"]
    ET = ECP // 128
    ECH = ECP // 512
    NW = NPAD // 128
    f = lambda k: np.asarray(inputs[k], np.float32)
    b = lambda a: np.ascontiguousarray(a).astype(np.float16)

    ei = np.asarray(inputs["edge_index"])
    src_g, dst_g = ei[0].astype(np.int64), ei[1].astype(np.int64)
    ea = f("edge_attr")
    x = f("x")
    cnt = np.bincount(dst_g, minlength=N).astype(np.float32)
    icnt_full = 1.0 / np.maximum(cnt, 1.0)

    NS1, NS2 = cfg["NS1"], cfg["NS2"]
    def tblrow2(g):
        c = g // NPC
        r = g % NPC
        return np.where(r < NS1, c * NS1 + r,
               np.where(r < NS2, NC * NS1 + c * (NS2 - NS1) + (r - NS1),
                        NC * NS2 + c * (NPAD - NS2) + (r - NS2)))

    order = np.argsort(dst_g, kind="stable")
    pos = {}
    for t, wd in enumerate(wsched):
        pos.setdefault(wd, []).append(t)

    in_maps = []
    shared = None
    for c in range(NC):
        lo, hi = c * NPC, (c + 1) * NPC
        sel = order[(dst_g[order] >= lo) & (dst_g[order] < hi)]
        dl = dst_g[sel] - lo
        win = dl // 128
        srcv = np.zeros(ECP, np.int64)
        eav = np.zeros((ECP, 3), np.float32)
        seg = np.zeros((ET, 128, 128), np.float32)
        for wd in range(NW):
            idxs = np.where(win == wd)[0]
            tiles = pos.get(wd, [])
            assert len(idxs) <= len(tiles) * 128, (c, wd, len(idxs), len(tiles))
            for k, i in enumerate(idxs):
                t = tiles[k // 128]
                r = k % 128
                g = t * 128 + r
                e_ = sel[i]
                srcv[g] = src_g[e_]
                eav[g] = ea[e_]
                seg[t, r, dl[i] - 128 * wd] = 1.0
        # seg2[c, p, 0:4] = seg tiles (edge-major), seg2[c, p, 4:8] = segT (node-major)
        seg2 = np.zeros((ECH, 128, 2, 4, 128), np.float32)
        for ch in range(ECH):
            for j in range(4):
                t = ch * 4 + j
                seg2[ch, :, 0, j, :] = seg[t]
                seg2[ch, :, 1, j, :] = seg[t].T
        icnt_c = np.ones((128, NW), np.float32)
        for t in range(NW):
            for p in range(128):
                r = t * 128 + p
                if r < NPC:
                    icnt_c[p, t] = icnt_full[lo + r]
        xT = np.zeros((5, NPAD), np.float32)
        xT[:, :NPC] = x[lo:hi].T
        eaT = eav.T.copy()
        m = {
            "xT": b(xT), "eaT": b(eaT),
            "srci": _wrap_idx(tblrow2(srcv).astype(np.int16)),
            "seg2": b(seg2.reshape(ECH, 128, 1024)),
            "icnt": icnt_c,
        }
        if shared is None:
            shared = {
                "ident": b(np.eye(128)),
                "identf": np.eye(128, dtype=np.float32),
                "identq": np.eye(128).astype(ml_dtypes.float8_e4m3),
                "ones1": np.ones((1, 128), np.float32),
                "onesK": np.ones((128, 1), np.float32),
                "encNW0": b(f("encN_W0")), "encNW": b(f("encN_Ws")),
                "encEW0": b(f("encE_W0")), "encEW": b(f("encE_Ws")),
                "eW0": b(f("eW0").reshape(L, 3, 128, 128)),
                "eWs0": b(f("eWs")[:, 0]), "eWs1": b(f("eWs")[:, 1]),
                "nW0": b(f("nW0").reshape(L, 2, 128, 128)),
                "nWs0": b(f("nWs")[:, 0]), "nWs1": b(f("nWs")[:, 1]),
                "decW": b(np.stack([f("dec_W0"), f("dec_Ws")[0], f("dec_Ws")[1]])),
                "decWl": b(f("dec_Wl")),
                "vE": b(f("eWs")[:, 1].sum(axis=2).T.copy()),
                "vN": b(f("nWs")[:, 1].sum(axis=2).T.copy()),
                "GE": b(np.einsum("lkf,lmf->lkm", f("eWs")[:, 1], f("eWs")[:, 1])),
                "GN": b(np.einsum("lkf,lmf->lkm", f("nWs")[:, 1], f("nWs")[:, 1])),
                "onesh": b(np.ones((128, 1), np.float32)),
                "encNb": f("encN_bs").T.copy(), "encEb": f("encE_bs").T.copy(),
                "eb": f("ebs").reshape(L * 3, 128).T.copy(),
                "nb": f("nbs").reshape(L * 3, 128).T.copy(),
                "decb": f("dec_bs").T.copy(),
            }
            flg = cfg["flags"]
            if flg["eln"]:
                shared["elnw"] = np.tile(f("elnw")[:, None, :], (1, 128, 1))
                shared["elnb"] = np.tile(f("elnb")[:, None, :], (1, 128, 1))
            if flg["nln"]:
                shared["nlnw"] = np.tile(f("nlnw")[:, None, :], (1, 128, 1))
                shared["nlnb"] = np.tile(f("nlnb")[:, None, :], (1, 128, 1))
            if flg["gln"]:
                shared["gNw"] = f("encN_lnw").reshape(128, 1).copy()
                shared["gNb"] = f("encN_lnb").reshape(128, 1).copy()
                shared["gEw"] = f("encE_lnw").reshape(128, 1).copy()
                shared["gEb"] = f("encE_lnb").reshape(128, 1).copy()
            if flg["decbl"]:
                shared["decbl"] = np.tile(f("dec_bl")[None, :], (128, 1))
        m.update(shared)
        in_maps.append(m)
    return in_maps


def make_cfg(inputs):
    N = np.asarray(inputs["x"]).shape[0]
    E = np.asarray(inputs["edge_index"]).shape[1]
    L = np.asarray(inputs["eW0"]).shape[0]
    NPC = N // NC
    NPAD = ((NPC + 127) // 128) * 128
    NW = NPAD // 128
    ei = np.asarray(inputs["edge_index"])
    dst = ei[1].astype(np.int64)
    tw = []
    for wd in range(NW):
        mx = 1
        for c in range(NC):
            lo = c * NPC
            nwin = int(((dst >= lo + wd * 128) & (dst < min(lo + (wd + 1) * 128, lo + NPC))).sum())
            mx = max(mx, (nwin + 127) // 128)
        tw.append(mx)
    wsched = []
    for wd in range(NW):
        wsched += [wd] * tw[wd]
    while (len(wsched) * 128) % 512:
        wsched.append(NW - 1)
    # graph-LN stats assume padded slots contribute exactly zero (MLP(0)=0)
    assert not np.any(np.asarray(inputs["encN_bs"])), "nonzero encoder bias unsupported"
    assert not np.any(np.asarray(inputs["encE_bs"])), "nonzero encoder bias unsupported"
    assert not np.any(np.asarray(inputs["ebs"])[:, 2]), "nonzero 3rd edge-MLP bias unsupported"
    assert not np.any(np.asarray(inputs["nbs"])[:, 2]), "nonzero 3rd node-MLP bias unsupported"
    flags = {
        "eln": bool(np.any(np.asarray(inputs["elnw"]) != 1) or np.any(np.asarray(inputs["elnb"]) != 0)),
        "nln": bool(np.any(np.asarray(inputs["nlnw"]) != 1) or np.any(np.asarray(inputs["nlnb"]) != 0)),
        "gln": bool(
            np.any(np.asarray(inputs["encN_lnw"]) != 1) or np.any(np.asarray(inputs["encN_lnb"]) != 0)
            or np.any(np.asarray(inputs["encE_lnw"]) != 1) or np.any(np.asarray(inputs["encE_lnb"]) != 0)
        ),
        "decbl": bool(np.any(np.asarray(inputs["dec_bl"]) != 0)),
    }
    NS = (NW // 2) * 128 if NW > 1 else NPAD
    ET_ = len(wsched)
    # window-group splits at ~30% / 70% of edge tiles
    cum = np.cumsum(tw)
    w1 = int(np.searchsorted(cum, 0.3 * ET_)) + 1
    w2 = int(np.searchsorted(cum, 0.7 * ET_)) + 1
    w1 = max(1, min(w1, NW - 2))
    w2 = max(w1 + 1, min(w2, NW - 1))
    return {
        "N": N, "E": E, "L": L, "NPC": NPC, "NPAD": NPAD,
        "EC_PAD": len(wsched) * 128, "wsched": wsched, "flags": flags, "NS": NS,
        "NS1": w1 * 128, "NS2": w2 * 128,
    }


_CACHE = {}


def kernel(**inputs) -> np.ndarray:
    cfg = make_cfg(inputs)
    key = (cfg["N"], cfg["E"], cfg["L"], cfg["EC_PAD"], tuple(sorted(cfg["flags"].items())))
    if key not in _CACHE:
        _CACHE[key] = build(cfg)
    nc = _CACHE[key]
    in_maps = _prep(inputs, cfg)
    res = run_bass_kernel_spmd(nc, in_maps, list(range(NC))).results
    NPC = cfg["NPC"]
    out = np.concatenate([res[c]["out"][:NPC] for c in range(NC)], axis=0)
    return out.astype(np.float32)



# revision 15
# speedup vs baseline: 1.0836x; 1.0836x over previous
import sys, os
sys.path.insert(0, "/opt/trn_rl_repo")
import numpy as np
import ml_dtypes

from concourse import bass, bacc, tile, mybir
from concourse.bass_utils import run_bass_kernel_spmd

bf16 = mybir.dt.float16  # fp16: same PE rate as bf16, 3 more mantissa bits
f32 = mybir.dt.float32
i16 = mybir.dt.int16
f8 = mybir.dt.float8e4
AF = mybir.ActivationFunctionType
ALU = mybir.AluOpType
X = mybir.AxisListType.X

NC = 8
H = 128
EPS = 1e-5
FP8_Z = False  # boundary z tables in fp8 (halves AllGather payload)


def _wrap_idx(a):
    # gather idx layout: token i at [i%16, i//16], replicated to 128 partitions
    n = len(a)
    n16 = (n + 15) // 16
    w = np.zeros((16, n16), np.int16)
    for p in range(16):
        w[p, : len(a[p::16])] = a[p::16]
    return np.tile(w, (8, 1))


def build(cfg):
    N, E, L = cfg["N"], cfg["E"], cfg["L"]
    NPC, NPAD, ECP = cfg["NPC"], cfg["NPAD"], cfg["EC_PAD"]
    wsched = cfg["wsched"]
    NT = NPAD // 128              # node tiles (== agg windows)
    ET = ECP // 128               # edge tiles
    ECH = ECP // 512              # edge chunks (4 tiles each)
    fl = cfg["flags"]
    KL = int(os.environ.get("KLAYERS", L))
    GB = cfg["GB"]                # window-group boundaries (rows), len NG, GB[-1]==NPAD
    NG = len(GB)
    gstart = [0] + GB[:-1]
    glen = [GB[g] - gstart[g] for g in range(NG)]
    TBL = NC * NPAD
    zdt = f8 if FP8_Z else bf16
    ZW = 256 if FP8_Z else 128    # z table row width (elems) for boundary tables

    # per-tile window + first/last-in-window flags
    fw = [False] * ET
    lw = [False] * ET
    seen = set()
    for t in range(ET):
        if wsched[t] not in seen:
            seen.add(wsched[t])
            fw[t] = True
    seen = set()
    for t in range(ET - 1, -1, -1):
        if wsched[t] not in seen:
            seen.add(wsched[t])
            lw[t] = True
    # last chunk index containing a tile of windows < nw
    def lastc(nw):
        return max(c for c in range(ECH)
                   if any(wsched[4 * c + j] < nw for j in range(4))) if nw > 0 else -1
    splitc = [lastc(GB[g] // 128) for g in range(NG - 1)]  # last group at loop end

    nc = bacc.Bacc(None, target_bir_lowering=False, num_devices=NC)
    P = lambda n_, s, d: nc.declare_dram_parameter(n_, s, d, isOutput=False)

    xT_d = P("xT", [5, NPAD], bf16)
    eaT_d = P("eaT", [3, ECP], bf16)
    src0_d = P("src0", [128, ECP // 16], i16)   # layer-0 table rows (c*NPAD+r)
    src4_d = P("src4", [128, ECP // 16], i16)   # group-major table rows
    seg2_d = P("seg2", [ECH, 128, 1024], bf16)
    ident_d = P("ident", [128, 128], bf16)
    identq_d = P("identq", [128, 128], f8)
    ones1_d = P("ones1", [1, 128], f32)
    encNW0_d = P("encNW0", [5, 128], bf16)
    encNW_d = P("encNW", [3, 128, 128], bf16)
    encEW0_d = P("encEW0", [3, 128], bf16)
    encEW_d = P("encEW", [3, 128, 128], bf16)
    eW0_d = P("eW0", [L, 3, 128, 128], bf16)
    eWs0_d = P("eWs0", [L, 128, 128], bf16)
    eWs1_d = P("eWs1", [L, 128, 128], bf16)
    nW0_d = P("nW0", [L, 2, 128, 128], bf16)
    nWs0_d = P("nWs0", [L, 128, 128], bf16)
    nWs1_d = P("nWs1", [L, 128, 128], bf16)
    decW_d = P("decW", [3, 128, 128], bf16)
    decWl_d = P("decWl", [128, 3], bf16)
    vE_d = P("vE", [128, L], bf16)
    vN_d = P("vN", [128, L], bf16)
    encNb_d = P("encNb", [128, 4], f32)
    encEb_d = P("encEb", [128, 4], f32)
    eb_d = P("eb", [128, 3 * L], f32)
    nb_d = P("nb", [128, 3 * L], f32)
    decb_d = P("decb", [128, 3], f32)
    if fl["eln"]:
        elnw_d = P("elnw", [L, 128, 128], f32)
        elnb_d = P("elnb", [L, 128, 128], f32)
    if fl["nln"]:
        nlnw_d = P("nlnw", [L, 128, 128], f32)
        nlnb_d = P("nlnb", [L, 128, 128], f32)
    if fl["gln"]:
        gNw_d = P("gNw", [128, 1], f32)
        gNb_d = P("gNb", [128, 1], f32)
        gEw_d = P("gEw", [128, 1], f32)
        gEb_d = P("gEb", [128, 1], f32)
    if fl["decbl"]:
        decbl_d = P("decbl", [128, 3], f32)

    out_d = nc.declare_dram_parameter("out", [NPAD, 3], f32, isOutput=True)
    zsh0_d = nc.dram_tensor("zsh0", [NPAD, 128], bf16)          # layer-0 shard
    zshg_d = [nc.dram_tensor(f"zshg{g}", [glen[g], 128], zdt) for g in range(NG)]
    ztab0_d = nc.dram_tensor("ztab0", [TBL, 128], bf16, addr_space="Shared")
    ztabA_d = nc.dram_tensor("ztabA", [TBL, ZW], zdt, addr_space="Shared")
    ztabB_d = nc.dram_tensor("ztabB", [TBL, ZW], zdt, addr_space="Shared")
    ztq = [ztabA_d, ztabB_d]
    sti_d = nc.dram_tensor("sti", [128], f32)
    sto_d = nc.dram_tensor("sto", [NC * 128], f32, addr_space="Shared")
    RG = [list(range(NC))]

    with tile.TileContext(nc) as tc:
        with (
            tc.tile_pool(name="const", bufs=1) as cp,
            tc.tile_pool(name="big", bufs=1) as bigp,
            tc.tile_pool(name="seg", bufs=3) as segp,
            tc.tile_pool(name="work", bufs=3) as wp,
            tc.tile_pool(name="stat", bufs=6) as sp,
            tc.tile_pool(name="pA", bufs=2, space="PSUM") as pA,
            tc.tile_pool(name="p3", bufs=2, space="PSUM") as p3,
            tc.tile_pool(name="pT", bufs=2, space="PSUM") as pT,
            tc.tile_pool(name="pG", bufs=2, space="PSUM") as pG,
        ):
            # ---- persistent SBUF ----
            e_fm = bigp.tile([128, ET, 128], bf16)      # e features, feature-major (fp16)
            h_own = bigp.tile([128, NT, 128], f32)      # node features f32, fm
            h_bf = bigp.tile([128, NT, 128], bf16)      # bf16 copy of h
            zdst_nm = bigp.tile([128, NT, 128], bf16)   # W1dst.T h, node-major
            agg_fm = bigp.tile([128, NT, 128], bf16)    # aggregated messages, fm
            accE = bigp.tile([128, ECH], f32)           # per-chunk e-enc sums
            accEq = bigp.tile([128, ECH], f32)
            NCH = (NPAD + 511) // 512
            accN = bigp.tile([128, NCH], f32)
            accNq = bigp.tile([128, NCH], f32)

            # ---- constants ----
            def ld(shape, dt, src, tag):
                t = cp.tile(shape, dt, tag=tag)
                nc.sync.dma_start(t[:], src[:])
                return t

            def ldw(src, n_, tag, dt=bf16):
                t = cp.tile([128, n_, 128], dt, tag=tag)
                nc.sync.dma_start(t[:], src[:].rearrange("n k m -> k n m"))
                return t

            xT = ld([5, NPAD], bf16, xT_d, "xT")
            srci0 = ld([128, ECP // 16], i16, src0_d, "srci0")
            srci4 = ld([128, ECP // 16], i16, src4_d, "srci4")
            ident = ld([128, 128], bf16, ident_d, "ident")
            identq = ld([128, 128], f8, identq_d, "identq")
            ones1 = ld([1, 128], f32, ones1_d, "ones1")
            encNW0 = ld([5, 128], bf16, encNW0_d, "encNW0")
            encNW = ldw(encNW_d, 3, "encNW")
            encEW0 = ld([3, 128], bf16, encEW0_d, "encEW0")
            encEW = ldw(encEW_d, 3, "encEW")
            eW0 = cp.tile([128, L * 3, 128], bf16, tag="eW0")
            nc.sync.dma_start(eW0[:], eW0_d[:].rearrange("l n k m -> k (l n) m"))
            eWs0 = ldw(eWs0_d, L, "eWs0")
            eWs1 = ldw(eWs1_d, L, "eWs1")
            nW0 = cp.tile([128, L * 2, 128], bf16, tag="nW0")
            nc.sync.dma_start(nW0[:], nW0_d[:].rearrange("l n k m -> k (l n) m"))
            nWs0 = ldw(nWs0_d, L, "nWs0")
            nWs1 = ldw(nWs1_d, L, "nWs1")
            decW = ldw(decW_d, 3, "decW")
            decWl = ld([128, 3], bf16, decWl_d, "decWl")
            vE = ld([128, L], bf16, vE_d, "vE")
            vN = ld([128, L], bf16, vN_d, "vN")
            encNb = ld([128, 4], f32, encNb_d, "encNb")
            encEb = ld([128, 4], f32, encEb_d, "encEb")
            eb = ld([128, 3 * L], f32, eb_d, "eb")
            nb = ld([128, 3 * L], f32, nb_d, "nb")
            decb = ld([128, 3], f32, decb_d, "decb")
            if fl["eln"]:
                elnw = ldw(elnw_d, L, "elnw", f32)
                elnb = ldw(elnb_d, L, "elnb", f32)
            if fl["nln"]:
                nlnw = ldw(nlnw_d, L, "nlnw", f32)
                nlnb = ldw(nlnb_d, L, "nlnb", f32)
            if fl["gln"]:
                gNw = ld([128, 1], f32, gNw_d, "gNw")
                gNb = ld([128, 1], f32, gNb_d, "gNb")
                gEw = ld([128, 1], f32, gEw_d, "gEw")
                gEb = ld([128, 1], f32, gEb_d, "gEb")
            if fl["decbl"]:
                decbl = ld([128, 3], f32, decbl_d, "decbl")
            epsA = sp.tile([128, 1], f32, tag="epsA")
            nc.vector.memset(epsA[:], EPS)

            # 4-linear encoder MLP, feature-major; returns final psum [128,w]
            # engines: a1 DVE, a2 Pool, a3 Act
            def enc_mlp(rhs, W0, Wm, b):
                w = rhs.shape[-1]
                ps = pA.tile([128, 512], f32, tag="pA")
                nc.tensor.matmul(ps[:, :w], W0, rhs, start=True, stop=True)
                a1 = wp.tile([128, 512], bf16, tag="a1")
                nc.vector.tensor_scalar(a1[:, :w], ps[:, :w], b[:, 0:1], 0.0, ALU.add, ALU.max)
                ps2 = p3.tile([128, 512], f32, tag="p3", name="ps2e")
                nc.tensor.matmul(ps2[:, :w], Wm[:, 0, :], a1[:, :w], start=True, stop=True)
                a2 = wp.tile([128, 512], bf16, tag="a2")
                nc.vector.tensor_scalar(a2[:, :w], ps2[:, :w], b[:, 1:2], 0.0, ALU.add, ALU.max)
                ps3 = pA.tile([128, 512], f32, tag="pA")
                nc.tensor.matmul(ps3[:, :w], Wm[:, 1, :], a2[:, :w], start=True, stop=True)
                a3 = wp.tile([128, 512], bf16, tag="a3")
                nc.scalar.activation(a3[:, :w], ps3[:, :w], AF.Relu, bias=b[:, 2:3])
                ps4 = p3.tile([128, 512], f32, tag="p3", name="ps4e")
                nc.tensor.matmul(ps4[:, :w], Wm[:, 2, :], a3[:, :w], start=True, stop=True)
                return ps4

            # LN row stats: s1 via per-tile matmul against weight-rowsum vcol (psS);
            # s2 via fused square+reduce (TTR). Returns rs (1/std) and nmr (-mu/std).
            def ln_stats(ysb, nt, a2t, vcol, psS):
                for j in range(nt):
                    nc.tensor.matmul(psS[:, j:j + 1], a2t[:, j * 128:(j + 1) * 128], vcol, start=True, stop=True)
                st2 = sp.tile([128, 4], f32, tag="st2")
                scr = wp.tile([128, 4, 128], bf16, tag="ttrscr")
                for j in range(nt):
                    nc.vector.scalar_tensor_tensor(scr[:, j, :], ysb[:, j, :], 1.0, ysb[:, j, :],
                                                   ALU.mult, ALU.mult, accum_out=st2[:, j:j + 1])
                st1 = sp.tile([128, 4], f32, tag="st1")
                nc.vector.tensor_copy(st1[:, 0:nt], psS[:, 0:nt])
                t2 = sp.tile([128, 4], f32, tag="mu2")
                nc.vector.tensor_tensor(t2[:, 0:nt], st1[:, 0:nt], st1[:, 0:nt], ALU.mult)
                t3 = sp.tile([128, 4], f32, tag="var")
                nc.vector.scalar_tensor_tensor(t3[:, 0:nt], t2[:, 0:nt], -1.0 / 128, st2[:, 0:nt], ALU.mult, ALU.add)
                sd = sp.tile([128, 4], f32, tag="sd")
                nc.scalar.activation(sd[:, 0:nt], t3[:, 0:nt], AF.Sqrt, bias=epsA[:], scale=1.0 / 128)
                rs = sp.tile([128, 4], f32, tag="rs")
                nc.vector.reciprocal(rs[:, 0:nt], sd[:, 0:nt])
                nmr = sp.tile([128, 4], f32, tag="nmr")
                nc.vector.scalar_tensor_tensor(nmr[:, 0:nt], st1[:, 0:nt], -1.0 / 128, rs[:, 0:nt], ALU.mult, ALU.mult)
                return rs, nmr

            # build z tables for layer l (weights eW0[3l], eW0[3l+1]) from h_bf rows
            # [t0*128, t0*128+nt*128); write zdst_nm + z shard DMA (dtype per target)
            def z_build(l, t0, nt, to_l0):
                pz = p3.tile([128, 4, 128], f32, tag="p3")
                pz2 = p3.tile([128, 4, 128], f32, tag="p3")
                for j in range(nt):
                    t = t0 + j
                    nc.tensor.matmul(pz[:, j, :], h_bf[:, t, :], eW0[:, 3 * l + 1, :], start=True, stop=True)
                    nc.tensor.matmul(pz2[:, j, :], h_bf[:, t, :], eW0[:, 3 * l + 0, :], start=True, stop=True)
                nc.vector.tensor_copy(zdst_nm[:, t0:t0 + nt, :], pz2[:, 0:nt, :])
                if to_l0:
                    zr = wp.tile([128, 4, 128], bf16, tag="zr")
                    nc.scalar.activation(zr[:].rearrange("p a b -> p (a b)")[:, :nt * 128],
                                         pz[:].rearrange("p a b -> p (a b)")[:, :nt * 128], AF.Copy)
                    for j in range(nt):
                        r0_ = (t0 + j) * 128
                        nc.sync.dma_start(zsh0_d[r0_:r0_ + 128, :], zr[:, j, :])
                else:
                    zr = wp.tile([128, 4, 128], zdt, tag="zrq")
                    nc.scalar.activation(zr[:].rearrange("p a b -> p (a b)")[:, :nt * 128],
                                         pz[:].rearrange("p a b -> p (a b)")[:, :nt * 128], AF.Copy)
                    for j in range(nt):
                        r0_ = (t0 + j) * 128
                        for g in range(NG):
                            if gstart[g] <= r0_ < GB[g]:
                                nc.sync.dma_start(zshg_d[g][r0_ - gstart[g]:r0_ - gstart[g] + 128, :], zr[:, j, :])

            # ================= NODE ENCODER =================
            for c in range(NCH):
                c0 = c * 512
                w = min(512, NPAD - c0)
                nt = w // 128
                ps4 = enc_mlp(xT[:, c0:c0 + w], encNW0[:], encNW, encNb)
                hv = h_own[:].rearrange("p a b -> p (a b)")[:, c0:c0 + w]
                nc.scalar.activation(hv, ps4[:, :w], AF.Copy, accum_out=accN[:, c:c + 1])
                hb = h_bf[:].rearrange("p a b -> p (a b)")[:, c0:c0 + w]
                nc.gpsimd.tensor_copy(hb, hv)
                scr = wp.tile([128, 512], bf16, tag="scr")
                nc.vector.scalar_tensor_tensor(scr[:, :w], hb, 1.0, hb, ALU.mult, ALU.mult,
                                               accum_out=accNq[:, c:c + 1])

            # z0 tables from raw h (pre graph-LN); valid when gln scale is uniform
            if not fl["gln"]:
                for c in range(NCH):
                    nt = min(4, NT - c * 4)
                    z_build(0, c * 4, nt, True)
                nc.gpsimd.collective_compute("AllGather", ALU.bypass, replica_groups=RG,
                                             ins=[zsh0_d[:]], outs=[ztab0_d[:]])

            # ================= EDGE ENCODER =================
            for c in range(ECH):
                c0 = c * 512
                eat = wp.tile([3, 512], bf16, tag="eat")
                nc.sync.dma_start(eat[:], eaT_d[:, c0:c0 + 512])
                ps4 = enc_mlp(eat[:], encEW0[:], encEW, encEb)
                ev = e_fm[:].rearrange("p a b -> p (a b)")[:, c0:c0 + 512]
                nc.scalar.activation(ev, ps4[:], AF.Copy, accum_out=accE[:, c:c + 1])
                scr = wp.tile([128, 512], bf16, tag="scr")
                nc.vector.scalar_tensor_tensor(scr[:], ev, 1.0, ev, ALU.mult, ALU.mult,
                                               accum_out=accEq[:, c:c + 1])

            # graph-stats AllGather (h + e sums); global reduce done locally
            st4 = sp.tile([128, 4], f32, tag="st4")
            nc.vector.tensor_reduce(st4[:, 0:1], accN[:, 0:NCH], X, ALU.add)
            nc.vector.tensor_reduce(st4[:, 1:2], accNq[:, 0:NCH], X, ALU.add)
            nc.vector.tensor_reduce(st4[:, 2:3], accE[:, 0:ECH], X, ALU.add)
            nc.vector.tensor_reduce(st4[:, 3:4], accEq[:, 0:ECH], X, ALU.add)
            onesK = sp.tile([128, 1], f32, tag="onesK")
            nc.vector.memset(onesK[:], 1.0)
            psst = p3.tile([128, 4, 128], f32, tag="p3", name="psst")
            nc.tensor.matmul(psst[:4, 0, :1], st4[:], onesK[:], start=True, stop=True)
            stv = sp.tile([4, 1], f32, tag="stv")
            nc.scalar.activation(stv[:], psst[:4, 0, :1], AF.Copy)
            nc.sync.dma_start(sti_d[0:4], stv[:, 0:1])
            nc.gpsimd.collective_compute("AllGather", ALU.bypass, replica_groups=RG,
                                         ins=[sti_d[:]], outs=[sto_d[:]])

            # graph-LN factors from gathered per-core stats:
            # a = w/(std+eps), c = b - mu*a  (w=1,b=0 unless gln)
            st32 = sp.tile([1, NC * 4], f32, tag="st32")
            nc.sync.dma_start(st32[:].rearrange("p (c k) -> p c k", k=4), sto_d[:].rearrange("(c k) -> c k", k=128)[:, 0:4])
            psb = p3.tile([128, 4, 128], f32, tag="p3", name="psb")
            nc.tensor.matmul(psb[:, 0, :NC * 4], ones1[:], st32[:], start=True, stop=True)
            stsum = sp.tile([128, 4], f32, tag="stsum")
            nc.vector.tensor_reduce(
                stsum[:], psb[:, 0, :NC * 4].rearrange("p (c k) -> p k c", k=4), X, ALU.add)

            def g_factors(s1col, s2col, count, gw, gb, tagp):
                mu = sp.tile([128, 1], f32, tag="gmu" + tagp)
                nc.vector.tensor_scalar(mu[:], s1col, 1.0 / count, None, ALU.mult)
                e2 = sp.tile([128, 1], f32, tag="ge2" + tagp)
                nc.vector.tensor_scalar(e2[:], s2col, 1.0 / count, None, ALU.mult)
                mu2 = sp.tile([128, 1], f32, tag="gmu2" + tagp)
                nc.vector.tensor_tensor(mu2[:], mu[:], mu[:], ALU.mult)
                var = sp.tile([128, 1], f32, tag="gvar" + tagp)
                nc.vector.tensor_tensor(var[:], e2[:], mu2[:], ALU.subtract)
                sd = sp.tile([128, 1], f32, tag="gsd" + tagp)
                nc.scalar.activation(sd[:], var[:], AF.Sqrt)
                nc.vector.tensor_scalar(sd[:], sd[:], EPS, None, ALU.add)
                a = sp.tile([128, 1], f32, tag="ga" + tagp)
                nc.vector.reciprocal(a[:], sd[:])
                if gw is not None:
                    nc.vector.tensor_tensor(a[:], a[:], gw[:], ALU.mult)
                cc = sp.tile([128, 1], f32, tag="gc" + tagp)
                nc.vector.tensor_scalar(cc[:], mu[:], a[:], -1.0, ALU.mult, ALU.mult)
                if gb is not None:
                    nc.vector.tensor_tensor(cc[:], cc[:], gb[:], ALU.add)
                return a, cc

            a_h, c_h = g_factors(stsum[:, 0:1], stsum[:, 1:2], float(N) * H,
                                 gNw if fl["gln"] else None, gNb if fl["gln"] else None, "h")
            a_e, c_e = g_factors(stsum[:, 2:3], stsum[:, 3:4], float(E) * H,
                                 gEw if fl["gln"] else None, gEb if fl["gln"] else None, "e")

            # apply h graph-LN; refresh h_bf
            for t in range(NT):
                nc.vector.tensor_scalar(h_own[:, t, :], h_own[:, t, :], a_h[:], c_h[:], ALU.mult, ALU.add)
                nc.gpsimd.tensor_copy(h_bf[:, t, :], h_own[:, t, :])
            if fl["gln"]:
                # gln: rebuild z tables from normalized h (late allgather)
                for c in range(NCH):
                    nt = min(4, NT - c * 4)
                    z_build(0, c * 4, nt, True)
                nc.gpsimd.collective_compute("AllGather", ALU.bypass, replica_groups=RG,
                                             ins=[zsh0_d[:]], outs=[ztab0_d[:]])

            # layer-0 graph-LN fold (all ReLU-homogeneous in a_h):
            #   ps1' = zdst_raw + zsrc_raw + (a_e/a_h) W1e.T e_raw
            #   a1'  = Relu(ps1' + b1'),  b1' = (eb0 + W1e.T c_e + (W1s+W1d).T c_h)/a_h
            #   ps2  = (a_h eWs0).T a1'  -> true scale from here on
            # gln path rebuilds z from normalized h: plain fold a_e into W1e only
            r_ah = sp.tile([128, 1], f32, tag="r_ah")
            nc.vector.reciprocal(r_ah[:], a_h[:])
            w1e0 = cp.tile([128, 128], bf16, tag="w1e0")
            if not fl["gln"]:
                aeoh = sp.tile([128, 1], f32, tag="aeoh")
                nc.vector.tensor_tensor(aeoh[:], a_e[:], r_ah[:], ALU.mult)
                nc.vector.tensor_scalar(w1e0[:], eW0[:, 2, :], aeoh[:], None, ALU.mult)
                w2e0 = cp.tile([128, 128], bf16, tag="w2e0")
                nc.vector.tensor_scalar(w2e0[:], eWs0[:, 0, :], a_h[:], None, ALU.mult)
            else:
                nc.vector.tensor_scalar(w1e0[:], eW0[:, 2, :], a_e[:], None, ALU.mult)
                w2e0 = None
            identae = cp.tile([128, 128], bf16, tag="identae")
            nc.vector.tensor_scalar(identae[:], ident[:], a_e[:], None, ALU.mult)
            ceb = sp.tile([128, 1], bf16, tag="ceb")
            nc.vector.tensor_copy(ceb[:], c_e[:])
            chb = sp.tile([128, 1], bf16, tag="chb")
            nc.vector.tensor_copy(chb[:], c_h[:])
            psb1 = p3.tile([128, 4, 128], f32, tag="p3", name="psb1")
            if not fl["gln"]:
                nc.tensor.matmul(psb1[:, 0, 0:1], eW0[:, 2, :], ceb[:], start=True, stop=False)
                nc.tensor.matmul(psb1[:, 0, 0:1], eW0[:, 0, :], chb[:], start=False, stop=False)
                nc.tensor.matmul(psb1[:, 0, 0:1], eW0[:, 1, :], chb[:], start=False, stop=True)
            else:
                nc.tensor.matmul(psb1[:, 0, 0:1], eW0[:, 2, :], ceb[:], start=True, stop=True)
            b1f0 = sp.tile([128, 1], f32, tag="b1f0")
            nc.vector.tensor_tensor(b1f0[:], eb[:, 0:1], psb1[:, 0, 0:1], ALU.add)
            if not fl["gln"]:
                nc.vector.tensor_tensor(b1f0[:], b1f0[:], r_ah[:], ALU.mult)

            # ================= DECODER =================
            def dec_part(r0, r1):
              c0 = r0
              while c0 < r1:
                w = min(512, r1 - c0)
                nt = w // 128
                hv = h_bf[:].rearrange("p a b -> p (a b)")[:, c0:c0 + w]
                ps = pA.tile([128, 512], f32, tag="pA")
                nc.tensor.matmul(ps[:, :w], decW[:, 0, :], hv, start=True, stop=True)
                a1 = wp.tile([128, 512], bf16, tag="a1")
                nc.scalar.activation(a1[:, :w], ps[:, :w], AF.Relu, bias=decb[:, 0:1])
                ps2 = pA.tile([128, 512], f32, tag="pA")
                nc.tensor.matmul(ps2[:, :w], decW[:, 1, :], a1[:, :w], start=True, stop=True)
                a2 = wp.tile([128, 512], bf16, tag="a2")
                nc.vector.tensor_scalar(a2[:, :w], ps2[:, :w], decb[:, 1:2], 0.0, ALU.add, ALU.max)
                ps3 = pA.tile([128, 512], f32, tag="pA")
                nc.tensor.matmul(ps3[:, :w], decW[:, 2, :], a2[:, :w], start=True, stop=True)
                a3 = wp.tile([128, 512], bf16, tag="a1")
                nc.scalar.activation(a3[:, :w], ps3[:, :w], AF.Relu, bias=decb[:, 2:3])
                pd = p3.tile([128, 4, 128], f32, tag="p3")
                for j in range(nt):
                    nc.tensor.matmul(pd[:, j, :3], a3[:, j * 128:(j + 1) * 128], decWl[:], start=True, stop=True)
                ot = wp.tile([128, 4, 3], f32, tag="ot")
                nc.vector.tensor_copy(ot[:, 0:nt, :], pd[:, 0:nt, 0:3])
                if fl["decbl"]:
                    for j in range(nt):
                        nc.vector.tensor_tensor(ot[:, j, :], ot[:, j, :], decbl[:], ALU.add)
                for j in range(nt):
                    t = c0 // 128 + j
                    nc.sync.dma_start(out_d[t * 128:(t + 1) * 128, :], ot[:, j, :])
                c0 += w

            # ================= MESSAGE-PASSING LAYERS =================
            def node_phase(l, r0, r1, last_layer):
                """process node rows [r0, r1); update h, build z_{l+1} tables"""
                c0 = r0
                while c0 < r1:
                    w = min(512, r1 - c0)
                    nt = w // 128
                    t0 = c0 // 128
                    hv = h_bf[:].rearrange("p a b -> p (a b)")[:, c0:c0 + w]
                    av = agg_fm[:].rearrange("p a b -> p (a b)")[:, c0:c0 + w]
                    ps = pA.tile([128, 512], f32, tag="pA")
                    nc.tensor.matmul(ps[:, :w], nW0[:, 2 * l, :], hv, start=True, stop=False)
                    nc.tensor.matmul(ps[:, :w], nW0[:, 2 * l + 1, :], av, start=False, stop=True)
                    a1 = wp.tile([128, 512], bf16, tag="a1")
                    nc.scalar.activation(a1[:, :w], ps[:, :w], AF.Relu, bias=nb[:, 3 * l:3 * l + 1])
                    ps2 = pA.tile([128, 512], f32, tag="pA")
                    nc.tensor.matmul(ps2[:, :w], nWs0[:, l, :], a1[:, :w], start=True, stop=True)
                    a2 = wp.tile([128, 512], bf16, tag="a2")
                    nc.scalar.activation(a2[:, :w], ps2[:, :w], AF.Relu, bias=nb[:, 3 * l + 1:3 * l + 2])
                    ps3 = p3.tile([128, 4, 128], f32, tag="p3")
                    for j in range(nt):
                        nc.tensor.matmul(ps3[:, j, :], a2[:, j * 128:(j + 1) * 128], nWs1[:, l, :], start=True, stop=True)
                    ysb = wp.tile([128, 4, 128], bf16, tag="ysb")
                    nc.vector.tensor_copy(ysb[:].rearrange("p a b -> p (a b)")[:, :w],
                                          ps3[:].rearrange("p a b -> p (a b)")[:, :w])
                    rs, nmr = ln_stats(ysb, nt, a2[:], vN[:, l:l + 1], ps2[:, 0:4])
                    updb = wp.tile([128, 4, 128], bf16, tag="updb")
                    for j in range(nt):
                        nc.vector.tensor_scalar(updb[:, j, :], ysb[:, j, :], rs[:, j:j + 1],
                                                nmr[:, j:j + 1], ALU.mult, ALU.add)
                        if fl["nln"]:
                            nc.vector.tensor_tensor(updb[:, j, :], updb[:, j, :], nlnw[:, l, :], ALU.mult)
                            nc.vector.tensor_tensor(updb[:, j, :], updb[:, j, :], nlnb[:, l, :], ALU.add)
                    ptn = pT.tile([128, 4, 128], f32, tag="pT", name="ptn")
                    for j in range(nt):
                        nc.tensor.matmul(ptn[:, j, :], updb[:, j, :], ident[:], start=True, stop=True)
                    hvo = h_own[:].rearrange("p a b -> p (a b)")[:, c0:c0 + w]
                    nc.vector.tensor_tensor(hvo, hvo, ptn[:].rearrange("p a b -> p (a b)")[:, :w], ALU.add)
                    nc.gpsimd.tensor_copy(hv, hvo)
                    if not last_layer:
                        z_build(l + 1, t0, nt, False)
                    c0 += w

            aggps = {}
            for l in range(KL):
                lay0 = (l == 0) and not fl["gln"]
                for c in range(ECH):
                    c0 = c * 512
                    s2c = segp.tile([128, 2, 4, 128], bf16, tag="s2c")
                    nc.sync.dma_start(s2c[:], seg2_d[c].rearrange("p (a b m) -> p a b m", a=2, b=4))
                    if l == 0:
                        zg = wp.tile([128, 4, 128], bf16, tag="zg", bufs=12)
                        nc.gpsimd.dma_gather(zg[:], ztab0_d[:], srci0[:, c * 32:(c + 1) * 32],
                                             512, 512, 128, transpose=False)
                        zgsl = [zg[:, j, :] for j in range(4)]
                        zid = ident
                    else:
                        zg = wp.tile([128, 4, ZW], zdt, tag="zgq", bufs=12)
                        nc.gpsimd.dma_gather(zg[:], ztq[l % 2][:], srci4[:, c * 32:(c + 1) * 32],
                                             512, 512, ZW, transpose=False)
                        zgsl = [zg[:, j, 0:128] for j in range(4)]
                        zid = identq if FP8_Z else ident

                    # ps1 = W1e.T e  +  zdst  +  zsrc
                    ps = pA.tile([128, 512], f32, tag="pA")
                    w1e = w1e0 if l == 0 else eW0[:, 3 * l + 2, :]
                    for j in range(4):
                        t = 4 * c + j
                        psj = ps[:, j * 128:(j + 1) * 128]
                        nc.tensor.matmul(psj, w1e, e_fm[:, 4 * c + j, :], start=True, stop=False)
                        nc.tensor.matmul(psj, zdst_nm[:, wsched[t], :], s2c[:, 1, j, :], start=False, stop=False)
                        nc.tensor.matmul(psj, zgsl[j], zid[:], start=False, stop=True)
                    a1 = wp.tile([128, 512], bf16, tag="a1")
                    nc.scalar.activation(a1[:], ps[:], AF.Relu,
                                         bias=b1f0[:] if l == 0 else eb[:, 3 * l:3 * l + 1])
                    ps2 = pA.tile([128, 512], f32, tag="pA")
                    w2e = w2e0 if lay0 else eWs0[:, l, :]
                    nc.tensor.matmul(ps2[:], w2e, a1[:], start=True, stop=True)
                    a2 = wp.tile([128, 512], bf16, tag="a2")
                    nc.scalar.activation(a2[:], ps2[:], AF.Relu, bias=eb[:, 3 * l + 1:3 * l + 2])
                    ps3 = p3.tile([128, 4, 128], f32, tag="p3")
                    for j in range(4):
                        nc.tensor.matmul(ps3[:, j, :], a2[:, j * 128:(j + 1) * 128], eWs1[:, l, :], start=True, stop=True)
                    ysb = wp.tile([128, 4, 128], bf16, tag="ysb")
                    nc.vector.tensor_copy(ysb[:].rearrange("p a b -> p (a b)"),
                                          ps3[:].rearrange("p a b -> p (a b)"))
                    rs, nmr = ln_stats(ysb, 4, a2[:], vE[:, l:l + 1], ps2[:, 0:4])
                    tmpb = wp.tile([128, 4, 128], bf16, tag="tmpb")
                    for j in range(4):
                        nc.vector.tensor_scalar(tmpb[:, j, :], ysb[:, j, :], rs[:, j:j + 1],
                                                nmr[:, j:j + 1], ALU.mult, ALU.add)
                        if fl["eln"]:
                            nc.vector.tensor_tensor(tmpb[:, j, :], tmpb[:, j, :], elnw[:, l, :], ALU.mult)
                            nc.vector.tensor_tensor(tmpb[:, j, :], tmpb[:, j, :], elnb[:, l, :], ALU.add)
                    # scatter into window psum accumulators (icnt folded into seg)
                    for j in range(4):
                        t = 4 * c + j
                        wd = wsched[t]
                        if fw[t]:
                            aggps[wd] = pG.tile([128, 128], f32, tag="pG", name="aggw")
                        nc.tensor.matmul(aggps[wd][:], s2c[:, 0, j, :], tmpb[:, j, :],
                                         start=fw[t], stop=lw[t])
                        if lw[t]:
                            agf = wp.tile([128, 128], bf16, tag="agf")
                            nc.vector.tensor_copy(agf[:], aggps[wd][:])
                            pag = pT.tile([128, 4, 128], f32, tag="pT", name="pag")
                            nc.tensor.matmul(pag[:, 0, :], agf[:], ident[:], start=True, stop=True)
                            nc.scalar.activation(agg_fm[:, wd, :], pag[:, 0, :], AF.Copy)
                            del aggps[wd]
                    # residual: ptr = tmp^T + (a_e*)e_old accumulated in psum, then one copy
                    ptr = pT.tile([128, 4, 128], f32, tag="pT")
                    eidt = identae if l == 0 else ident
                    for j in range(4):
                        nc.tensor.matmul(ptr[:, j, :], tmpb[:, j, :], ident[:], start=True, stop=False)
                        nc.tensor.matmul(ptr[:, j, :], eidt[:], e_fm[:, 4 * c + j, :], start=False, stop=True)
                    ev = e_fm[:].rearrange("p a b -> p (a b)")[:, c0:c0 + 512]
                    if l == 0:
                        nc.scalar.activation(ev, ptr[:].rearrange("p a b -> p (a b)"), AF.Identity, bias=c_e[:])
                    else:
                        nc.scalar.activation(ev, ptr[:].rearrange("p a b -> p (a b)"), AF.Copy)
                    # pipelined node phases at window-group boundaries;
                    # AGs issued promptly (collective seq-wait parks Pool briefly)
                    if KL == L:
                        for g in range(NG - 1):
                            if c == splitc[g]:
                                node_phase(l, gstart[g], GB[g], l == L - 1)
                            if c == min(splitc[g] + 1, ECH - 1) and l < L - 1:
                                zt_out = (ztq[(l + 1) % 2][NC * gstart[g]:NC * GB[g], 0:128]
                                          if FP8_Z else ztq[(l + 1) % 2][NC * gstart[g]:NC * GB[g]])
                                nc.gpsimd.collective_compute(
                                    "AllGather", ALU.bypass, replica_groups=RG,
                                    ins=[zshg_d[g][:]], outs=[zt_out])
                if KL == L:
                    g = NG - 1
                    node_phase(l, gstart[g], NPAD, l == L - 1)
                    if l < L - 1:
                        zt_out = (ztq[(l + 1) % 2][NC * gstart[g]:NC * GB[g], 0:128]
                                  if FP8_Z else ztq[(l + 1) % 2][NC * gstart[g]:NC * GB[g]])
                        nc.gpsimd.collective_compute(
                            "AllGather", ALU.bypass, replica_groups=RG,
                            ins=[zshg_d[g][:]], outs=[zt_out])
                else:
                    node_phase(l, 0, NPAD, l == KL - 1)

            dec_part(0, NPAD)


    nc.compile()
    return nc


def _prep(inputs, cfg):
    N, E, L = cfg["N"], cfg["E"], cfg["L"]
    NPC, NPAD, ECP = cfg["NPC"], cfg["NPAD"], cfg["EC_PAD"]
    wsched = cfg["wsched"]
    GB = cfg["GB"]
    NG = len(GB)
    gstart = [0] + GB[:-1]
    ET = ECP // 128
    ECH = ECP // 512
    NW = NPAD // 128
    f = lambda k: np.asarray(inputs[k], np.float32)
    b = lambda a: np.ascontiguousarray(a).astype(np.float16)

    ei = np.asarray(inputs["edge_index"])
    src_g, dst_g = ei[0].astype(np.int64), ei[1].astype(np.int64)
    ea = f("edge_attr")
    x = f("x")
    cnt = np.bincount(dst_g, minlength=N).astype(np.float32)
    icnt_full = 1.0 / np.maximum(cnt, 1.0)

    def tblrow0(g):
        c = g // NPC
        r = g % NPC
        return c * NPAD + r

    def tblrow4(g):
        c = g // NPC
        r = g % NPC
        out = np.zeros_like(g)
        for gi in range(NG):
            m = (r >= gstart[gi]) & (r < GB[gi])
            out = np.where(m, NC * gstart[gi] + c * (GB[gi] - gstart[gi]) + (r - gstart[gi]), out)
        return out

    order = np.argsort(dst_g, kind="stable")
    pos = {}
    for t, wd in enumerate(wsched):
        pos.setdefault(wd, []).append(t)

    in_maps = []
    shared = None
    for c in range(NC):
        lo, hi = c * NPC, (c + 1) * NPC
        sel = order[(dst_g[order] >= lo) & (dst_g[order] < hi)]
        dl = dst_g[sel] - lo
        win = dl // 128
        srcv = np.zeros(ECP, np.int64)
        eav = np.zeros((ECP, 3), np.float32)
        seg = np.zeros((ET, 128, 128), np.float32)
        segi = np.zeros((ET, 128, 128), np.float32)  # icnt-scaled for agg
        for wd in range(NW):
            idxs = np.where(win == wd)[0]
            tiles = pos.get(wd, [])
            assert len(idxs) <= len(tiles) * 128, (c, wd, len(idxs), len(tiles))
            for k, i in enumerate(idxs):
                t = tiles[k // 128]
                r = k % 128
                g = t * 128 + r
                e_ = sel[i]
                srcv[g] = src_g[e_]
                eav[g] = ea[e_]
                seg[t, r, dl[i] - 128 * wd] = 1.0
                segi[t, r, dl[i] - 128 * wd] = icnt_full[lo + dl[i]]
        # seg2[c, p, 0] = icnt-scaled seg (edge-major), seg2[c, p, 1] = segT (node-major)
        seg2 = np.zeros((ECH, 128, 2, 4, 128), np.float32)
        for ch in range(ECH):
            for j in range(4):
                t = ch * 4 + j
                seg2[ch, :, 0, j, :] = segi[t]
                seg2[ch, :, 1, j, :] = seg[t].T
        xT = np.zeros((5, NPAD), np.float32)
        xT[:, :NPC] = x[lo:hi].T
        eaT = eav.T.copy()
        m = {
            "xT": b(xT), "eaT": b(eaT),
            "src0": _wrap_idx(tblrow0(srcv).astype(np.int16)),
            "src4": _wrap_idx(tblrow4(srcv).astype(np.int16)),
            "seg2": b(seg2.reshape(ECH, 128, 1024)),
        }
        if shared is None:
            shared = {
                "ident": b(np.eye(128)),
                "identq": np.eye(128).astype(ml_dtypes.float8_e4m3),
                "ones1": np.ones((1, 128), np.float32),
                "encNW0": b(f("encN_W0")), "encNW": b(f("encN_Ws")),
                "encEW0": b(f("encE_W0")), "encEW": b(f("encE_Ws")),
                "eW0": b(f("eW0").reshape(L, 3, 128, 128)),
                "eWs0": b(f("eWs")[:, 0]), "eWs1": b(f("eWs")[:, 1]),
                "nW0": b(f("nW0").reshape(L, 2, 128, 128)),
                "nWs0": b(f("nWs")[:, 0]), "nWs1": b(f("nWs")[:, 1]),
                "decW": b(np.stack([f("dec_W0"), f("dec_Ws")[0], f("dec_Ws")[1]])),
                "decWl": b(f("dec_Wl")),
                "vE": b(f("eWs")[:, 1].sum(axis=2).T.copy()),
                "vN": b(f("nWs")[:, 1].sum(axis=2).T.copy()),
                "encNb": f("encN_bs").T.copy(), "encEb": f("encE_bs").T.copy(),
                "eb": f("ebs").reshape(L * 3, 128).T.copy(),
                "nb": f("nbs").reshape(L * 3, 128).T.copy(),
                "decb": f("dec_bs").T.copy(),
            }
            flg = cfg["flags"]
            if flg["eln"]:
                shared["elnw"] = np.tile(f("elnw")[:, None, :], (1, 128, 1))
                shared["elnb"] = np.tile(f("elnb")[:, None, :], (1, 128, 1))
            if flg["nln"]:
                shared["nlnw"] = np.tile(f("nlnw")[:, None, :], (1, 128, 1))
                shared["nlnb"] = np.tile(f("nlnb")[:, None, :], (1, 128, 1))
            if flg["gln"]:
                shared["gNw"] = f("encN_lnw").reshape(128, 1).copy()
                shared["gNb"] = f("encN_lnb").reshape(128, 1).copy()
                shared["gEw"] = f("encE_lnw").reshape(128, 1).copy()
                shared["gEb"] = f("encE_lnb").reshape(128, 1).copy()
            if flg["decbl"]:
                shared["decbl"] = np.tile(f("dec_bl")[None, :], (128, 1))
        m.update(shared)
        in_maps.append(m)
    return in_maps


def make_cfg(inputs):
    N = np.asarray(inputs["x"]).shape[0]
    E = np.asarray(inputs["edge_index"]).shape[1]
    L = np.asarray(inputs["eW0"]).shape[0]
    NPC = N // NC
    NPAD = ((NPC + 127) // 128) * 128
    NW = NPAD // 128
    ei = np.asarray(inputs["edge_index"])
    dst = ei[1].astype(np.int64)
    tw = []
    for wd in range(NW):
        mx = 1
        for c in range(NC):
            lo = c * NPC
            nwin = int(((dst >= lo + wd * 128) & (dst < min(lo + (wd + 1) * 128, lo + NPC))).sum())
            mx = max(mx, (nwin + 127) // 128)
        tw.append(mx)
    wsched = []
    for wd in range(NW):
        wsched += [wd] * tw[wd]
    while (len(wsched) * 128) % 512:
        wsched.append(NW - 1)
    # graph-LN stats assume padded slots contribute exactly zero (MLP(0)=0)
    assert not np.any(np.asarray(inputs["encN_bs"])), "nonzero encoder bias unsupported"
    assert not np.any(np.asarray(inputs["encE_bs"])), "nonzero encoder bias unsupported"
    assert not np.any(np.asarray(inputs["ebs"])[:, 2]), "nonzero 3rd edge-MLP bias unsupported"
    assert not np.any(np.asarray(inputs["nbs"])[:, 2]), "nonzero 3rd node-MLP bias unsupported"
    flags = {
        "eln": bool(np.any(np.asarray(inputs["elnw"]) != 1) or np.any(np.asarray(inputs["elnb"]) != 0)),
        "nln": bool(np.any(np.asarray(inputs["nlnw"]) != 1) or np.any(np.asarray(inputs["nlnb"]) != 0)),
        "gln": bool(
            np.any(np.asarray(inputs["encN_lnw"]) != 1) or np.any(np.asarray(inputs["encN_lnb"]) != 0)
            or np.any(np.asarray(inputs["encE_lnw"]) != 1) or np.any(np.asarray(inputs["encE_lnb"]) != 0)
        ),
        "decbl": bool(np.any(np.asarray(inputs["dec_bl"]) != 0)),
    }
    ET_ = len(wsched)
    # 4 window-group splits at ~30/60/80/100% of edge tiles
    cum = np.cumsum(tw)
    ws = []
    prev = 0
    for frac in (0.3, 0.6, 0.8):
        w = int(np.searchsorted(cum, frac * ET_)) + 1
        w = max(prev + 1, min(w, NW - (3 - len(ws))))
        ws.append(w)
        prev = w
    GB = [w * 128 for w in ws] + [NPAD]
    return {
        "N": N, "E": E, "L": L, "NPC": NPC, "NPAD": NPAD,
        "EC_PAD": len(wsched) * 128, "wsched": wsched, "flags": flags,
        "GB": GB,
    }


_CACHE = {}


def kernel(**inputs) -> np.ndarray:
    cfg = make_cfg(inputs)
    key = (cfg["N"], cfg["E"], cfg["L"], cfg["EC_PAD"], tuple(sorted(cfg["flags"].items())))
    if key not in _CACHE:
        _CACHE[key] = build(cfg)
    nc = _CACHE[key]
    in_maps = _prep(inputs, cfg)
    res = run_bass_kernel_spmd(nc, in_maps, list(range(NC))).results
    NPC = cfg["NPC"]
    out = np.concatenate([res[c]["out"][:NPC] for c in range(NC)], axis=0)
    return out.astype(np.float32)
